# revision 16
# baseline (speedup 1.0000x reference)
"""Transformer block (pre-LN, 12-head attention + GELU MLP) on 8 TRN2 NeuronCores.

Sharding: pure data-parallel — batch 8 -> one sequence [1024, 768] per core,
no collectives. On-device everything is computed in "feature-major" layout
(features on partitions, tokens on the free axis) so no on-device transposes
are needed anywhere:

  - x arrives host-pre-transposed as xT [768, 1024] f32 (the f32 residual spine).
  - LayerNorm stats (over features = partitions) via ones-vector matmuls on PE;
    normalization applied with partition-broadcast rows on DVE.
  - qkv produces Q^T/K^T feature-major and V token-major directly; scores are
    computed as S^T = K^T.T-free matmuls (contraction over head dim), softmax
    without max-subtraction (logits are bounded ~|S|<4); the 1/8 scale is
    folded into exp's affine pre-scale; the softmax denominator comes from a
    ones-column appended to V, and normalization is applied to O^T (64 rows)
    fused into the PSUM->SBUF copy.
  - All matmul inputs in bf16 (weights pre-cast on host), f32 PSUM accumulate.
  - Output written feature-major, transposed back on host.
"""

import numpy as np
import ml_dtypes

import concourse.bass as bass
import concourse.tile as tile
from concourse import bacc, mybir
from concourse.bass import ts

F32 = mybir.dt.float32
BF16 = mybir.dt.bfloat16
AF = mybir.ActivationFunctionType
ALU = mybir.AluOpType

T = 1024          # tokens per core
C = 768           # model dim
H = 12            # heads
DH = 64           # head dim
FF = 3072         # mlp hidden
CK = C // 128     # 6 feature chunks
FK = FF // 128    # 24 hidden chunks
TK = T // 128     # 8 token tiles
NT = T // 512     # 2 psum-width chunks
EPS = 1e-5
N_CORES = 8

bf16 = ml_dtypes.bfloat16


def _emit_layernorm(nc, tc, ctx_pools, src, dst, lnw_sb, lnb_sb, ones_sb, eps_sb):
    """src: [128, CK, T] f32 sbuf tile; dst: [128, CK, T] bf16 sbuf tile.
    lnw_sb/lnb_sb: [128, CK] f32 (per-partition scalar columns per chunk)."""
    cast_pool, stat_pool, bc_pool = ctx_pools
    with tc.tile_pool(name="ln_psum", bufs=1, space="PSUM") as pst:
        sum_ps = pst.tile([1, T], F32, tag="sum")
        ssq_ps = pst.tile([1, T], F32, tag="ssq")
        for j in range(CK):
            xb = cast_pool.tile([128, T], BF16, tag="xb")
            nc.vector.tensor_copy(xb[:], src[:, j, :])
            sq = cast_pool.tile([128, T], BF16, tag="sq")
            nc.vector.tensor_mul(sq[:], xb[:], xb[:])
            for n in range(NT):
                nc.tensor.matmul(sum_ps[:, ts(n, 512)], ones_sb[:], xb[:, ts(n, 512)],
                                 start=(j == 0), stop=(j == CK - 1))
                nc.tensor.matmul(ssq_ps[:, ts(n, 512)], ones_sb[:], sq[:, ts(n, 512)],
                                 start=(j == 0), stop=(j == CK - 1))
        # stats on partition 0 rows [1, T]
        mu = stat_pool.tile([1, T], F32, tag="mu")
        nc.scalar.mul(mu[:], sum_ps[:], 1.0 / C)
        e2 = stat_pool.tile([1, T], F32, tag="e2")
        nc.scalar.mul(e2[:], ssq_ps[:], 1.0 / C)
    var = stat_pool.tile([1, T], F32, tag="var")
    nc.vector.tensor_mul(var[:], mu[:], mu[:])       # mu^2
    nc.vector.tensor_sub(var[:], e2[:], var[:])      # E[x^2] - mu^2
    nc.scalar.activation(var[:], var[:], AF.Sqrt, bias=eps_sb[:])  # std, in place
    rstd = stat_pool.tile([1, T], F32, tag="rstd")
    nc.vector.reciprocal(rstd[:], var[:])
    nc.vector.tensor_mul(mu[:], mu[:], rstd[:])      # mu * rstd, in place
    s_bc = bc_pool.tile([128, T], F32, tag="s_bc")
    nc.gpsimd.partition_broadcast(s_bc[:], rstd[:])
    m_bc = bc_pool.tile([128, T], F32, tag="m_bc")
    nc.gpsimd.partition_broadcast(m_bc[:], mu[:])
    for j in range(CK):
        t1 = cast_pool.tile([128, T], F32, tag="n1")
        nc.vector.tensor_mul(t1[:], src[:, j, :], s_bc[:])   # x * rstd
        nc.vector.tensor_sub(t1[:], t1[:], m_bc[:])          # - mu*rstd
        nc.vector.tensor_scalar(dst[:, j, :], t1[:],
                                lnw_sb[:, j:j + 1], lnb_sb[:, j:j + 1],
                                op0=ALU.mult, op1=ALU.add)


def _build():
    nc = bacc.Bacc("TRN2", target_bir_lowering=False, debug=False)

    xT = nc.dram_tensor("xT", [C, T], F32, kind="ExternalInput").ap()
    qkv_w = nc.dram_tensor("qkv_w", [C, 3 * C], BF16, kind="ExternalInput").ap()
    qkv_b = nc.dram_tensor("qkv_b", [3 * C], F32, kind="ExternalInput").ap()
    proj_w = nc.dram_tensor("proj_w", [C, C], BF16, kind="ExternalInput").ap()
    proj_b = nc.dram_tensor("proj_b", [C], F32, kind="ExternalInput").ap()
    fc1_w = nc.dram_tensor("fc1_w", [C, FF], BF16, kind="ExternalInput").ap()
    fc1_b = nc.dram_tensor("fc1_b", [FF], F32, kind="ExternalInput").ap()
    fc2_w = nc.dram_tensor("fc2_w", [FF, C], BF16, kind="ExternalInput").ap()
    fc2_b = nc.dram_tensor("fc2_b", [C], F32, kind="ExternalInput").ap()
    ln1_w = nc.dram_tensor("ln1_w", [C], F32, kind="ExternalInput").ap()
    ln1_b = nc.dram_tensor("ln1_b", [C], F32, kind="ExternalInput").ap()
    ln2_w = nc.dram_tensor("ln2_w", [C], F32, kind="ExternalInput").ap()
    ln2_b = nc.dram_tensor("ln2_b", [C], F32, kind="ExternalInput").ap()
    gamma1 = nc.dram_tensor("gamma1", [C], F32, kind="ExternalInput").ap()
    gamma2 = nc.dram_tensor("gamma2", [C], F32, kind="ExternalInput").ap()
    outT = nc.dram_tensor("outT", [C, T], F32, kind="ExternalOutput").ap()

    with tile.TileContext(nc) as tc:
        import contextlib
        with contextlib.ExitStack() as ctx:
            spine = ctx.enter_context(tc.tile_pool(name="spine", bufs=1))
            hTp = ctx.enter_context(tc.tile_pool(name="hTp", bufs=1))
            vecs = ctx.enter_context(tc.tile_pool(name="vecs", bufs=1))
            consts = ctx.enter_context(tc.tile_pool(name="consts", bufs=1))
            cast_pool = ctx.enter_context(tc.tile_pool(name="cast", bufs=2))
            stat_pool = ctx.enter_context(tc.tile_pool(name="stat", bufs=1))
            bc_pool = ctx.enter_context(tc.tile_pool(name="bc", bufs=1))
            ln_pools = (cast_pool, stat_pool, bc_pool)

            # ---- small persistent constants & vectors ----
            ones_sb = consts.tile([128, 1], BF16)
            nc.vector.memset(ones_sb[:], 1.0)
            eps_sb = consts.tile([1, 1], F32)
            nc.vector.memset(eps_sb[:], EPS)

            def load_vec(ap_1d, n):
                t = vecs.tile([128, n], F32, tag=f"v{ap_1d.tensor.name}")
                nc.sync.dma_start(t[:], ap_1d.rearrange("(j p) -> p j", p=128))
                return t

            ln1w_sb = load_vec(ln1_w, CK)
            ln1b_sb = load_vec(ln1_b, CK)
            ln2w_sb = load_vec(ln2_w, CK)
            ln2b_sb = load_vec(ln2_b, CK)
            g1_sb = load_vec(gamma1, CK)
            g2_sb = load_vec(gamma2, CK)
            qb_sb = load_vec(qkv_b, 3 * CK)     # [128, 18]
            pb_sb = load_vec(proj_b, CK)
            b1_sb = load_vec(fc1_b, FK)         # [128, 24]
            b2_sb = load_vec(fc2_b, CK)
            # v-bias broadcast along partitions: [128, C] f32
            vb_bc = vecs.tile([128, C], F32, tag="vb_bc")
            vb_src = bass.AP(tensor=qkv_b.tensor, offset=qkv_b.offset + 2 * C,
                             ap=[[0, 128]] + qkv_b[2 * C:3 * C].ap)
            nc.sync.dma_start(vb_bc[:], vb_src)

            # ---- load x spine (in-place evolves x -> x1 -> out) ----
            resid = spine.tile([128, CK, T], F32, tag="spine")
            for j in range(CK):
                nc.sync.dma_start(resid[:, j, :], xT[ts(j, 128), :])

            # ---- LN1 ----
            h1T = hTp.tile([128, CK, T], BF16, tag="hT")
            _emit_layernorm(nc, tc, ln_pools, resid, h1T, ln1w_sb, ln1b_sb, ones_sb, eps_sb)

            with contextlib.ExitStack() as actx:
                wsec = actx.enter_context(tc.tile_pool(name="wsec", bufs=2))
                qtp = actx.enter_context(tc.tile_pool(name="qtp", bufs=1))
                ktp = actx.enter_context(tc.tile_pool(name="ktp", bufs=1))
                vtp = actx.enter_context(tc.tile_pool(name="vtp", bufs=1))
                otp = actx.enter_context(tc.tile_pool(name="otp", bufs=1))
                esp = actx.enter_context(tc.tile_pool(name="esp", bufs=2))
                zip_ = actx.enter_context(tc.tile_pool(name="zip", bufs=1))
                zbp = actx.enter_context(tc.tile_pool(name="zbp", bufs=2))

                QT = qtp.tile([128, CK, T], BF16)
                KT = ktp.tile([128, CK, T], BF16)
                V = vtp.tile([128, TK, H, DH + 1], BF16)
                OT = otp.tile([128, CK, T], BF16)

                # ---- qkv matmuls ----
                with tc.tile_pool(name="qkv_ps", bufs=2, space="PSUM") as mmp:
                    for sec in range(2):  # 0 -> Q^T, 1 -> K^T  (feature-major)
                        w = wsec.tile([128, CK, C], BF16, tag="wsec")
                        for k in range(CK):
                            nc.sync.dma_start(w[:, k, :],
                                              qkv_w[ts(k, 128), sec * C:(sec + 1) * C])
                        dst = QT if sec == 0 else KT
                        for m in range(CK):
                            ps = mmp.tile([128, T], F32, tag="mm")
                            for k in range(CK):
                                for n in range(NT):
                                    nc.tensor.matmul(ps[:, ts(n, 512)],
                                                     w[:, k, ts(m, 128)],
                                                     h1T[:, k, ts(n, 512)],
                                                     start=(k == 0), stop=(k == CK - 1))
                            nc.vector.tensor_scalar(dst[:, m, :], ps[:],
                                                    qb_sb[:, sec * CK + m:sec * CK + m + 1],
                                                    None, op0=ALU.add)
                    # V token-major
                    w = wsec.tile([128, CK, C], BF16, tag="wsec")
                    for k in range(CK):
                        nc.sync.dma_start(w[:, k, :], qkv_w[ts(k, 128), 2 * C:3 * C])
                    nc.vector.memset(V[:, :, :, DH:DH + 1], 1.0)
                    for tk in range(TK):
                        ps = mmp.tile([128, C], F32, tag="vmm")
                        for k in range(CK):
                            nc.tensor.matmul(ps[:, 0:512], h1T[:, k, ts(tk, 128)],
                                             w[:, k, 0:512],
                                             start=(k == 0), stop=(k == CK - 1))
                            nc.tensor.matmul(ps[:, 512:C], h1T[:, k, ts(tk, 128)],
                                             w[:, k, 512:C],
                                             start=(k == 0), stop=(k == CK - 1))
                        nc.vector.tensor_add(
                            V[:, tk, :, 0:DH],
                            ps.rearrange("p (h d) -> p h d", d=DH),
                            vb_bc.rearrange("p (h d) -> p h d", d=DH))

                # ---- attention ----
                with tc.tile_pool(name="s_ps", bufs=2, space="PSUM") as spp, \
                     tc.tile_pool(name="o_ps", bufs=2, space="PSUM") as opp:
                    for h in range(H):
                        zq = 64 * (h % 2)
                        jq = h // 2
                        es = esp.tile([128, TK, T], BF16, tag="es")
                        for tk in range(TK):
                            sp = spp.tile([128, T], F32, tag="sp")
                            for n in range(NT):
                                nc.tensor.matmul(
                                    sp[:, ts(n, 512)],
                                    KT[zq:zq + 64, jq, ts(tk, 128)],
                                    QT[zq:zq + 64, jq, ts(n, 512)])
                            nc.scalar.activation(es[:, tk, :], sp[:], AF.Exp,
                                                 scale=float(DH) ** -0.5)
                        op = opp.tile([DH + 1, T], F32, tag="op")
                        for tk in range(TK):
                            for n in range(NT):
                                nc.tensor.matmul(op[:, ts(n, 512)],
                                                 V[:, tk, h, :],
                                                 es[:, tk, ts(n, 512)],
                                                 start=(tk == 0), stop=(tk == TK - 1))
                        zi = zip_.tile([1, T], F32, tag="zi")
                        nc.vector.reciprocal(zi[:], op[DH:DH + 1, :])
                        zbc = zbp.tile([64, T], F32, tag="zbc")
                        nc.gpsimd.partition_broadcast(zbc[:], zi[:])
                        nc.vector.tensor_mul(OT[zq:zq + 64, jq, :], op[0:DH, :], zbc[:])

                # ---- proj + residual -> x1 (in place on resid) ----
                w = wsec.tile([128, CK, C], BF16, tag="wsec")
                for k in range(CK):
                    nc.sync.dma_start(w[:, k, :], proj_w[ts(k, 128), :])
                with tc.tile_pool(name="pj_ps", bufs=2, space="PSUM") as pjp:
                    for m in range(CK):
                        ps = pjp.tile([128, T], F32, tag="pj")
                        for k in range(CK):
                            for n in range(NT):
                                nc.tensor.matmul(ps[:, ts(n, 512)],
                                                 w[:, k, ts(m, 128)],
                                                 OT[:, k, ts(n, 512)],
                                                 start=(k == 0), stop=(k == CK - 1))
                        # (ps + proj_b) * gamma1, then + xT
                        nc.vector.tensor_scalar(ps[:], ps[:],
                                                pb_sb[:, m:m + 1], g1_sb[:, m:m + 1],
                                                op0=ALU.add, op1=ALU.mult)
                        nc.vector.tensor_add(resid[:, m, :], ps[:], resid[:, m, :])

            # ---- LN2 ----
            h2T = hTp.tile([128, CK, T], BF16, tag="hT")
            _emit_layernorm(nc, tc, ln_pools, resid, h2T, ln2w_sb, ln2b_sb, ones_sb, eps_sb)

            # ---- MLP ----
            with contextlib.ExitStack() as mctx:
                w1p = mctx.enter_context(tc.tile_pool(name="w1p", bufs=1))
                gtp = mctx.enter_context(tc.tile_pool(name="gtp", bufs=1))
                w2p = mctx.enter_context(tc.tile_pool(name="w2p", bufs=2))

                gT = gtp.tile([128, FK, T], BF16)
                w1 = w1p.tile([128, CK, FF], BF16)
                for k in range(CK):
                    nc.sync.dma_start(w1[:, k, :], fc1_w[ts(k, 128), :])
                fc2_r = fc2_w.rearrange("(k p) c -> p k c", p=128)  # [128, FK, C]

                with tc.tile_pool(name="f1_ps", bufs=2, space="PSUM") as f1p, \
                     tc.tile_pool(name="f2_ps", bufs=2, space="PSUM") as f2p:
                    for m in range(FK):
                        ps = f1p.tile([128, T], F32, tag="f1")
                        for k in range(CK):
                            for n in range(NT):
                                nc.tensor.matmul(ps[:, ts(n, 512)],
                                                 w1[:, k, ts(m, 128)],
                                                 h2T[:, k, ts(n, 512)],
                                                 start=(k == 0), stop=(k == CK - 1))
                        nc.scalar.activation(gT[:, m, :], ps[:], AF.Gelu,
                                             bias=b1_sb[:, m:m + 1])
                    for m in range(CK):
                        w2 = w2p.tile([128, FK, 128], BF16, tag="w2")
                        nc.sync.dma_start(w2[:], fc2_r[:, :, ts(m, 128)])
                        ps = f2p.tile([128, T], F32, tag="f2")
                        for k in range(FK):
                            for n in range(NT):
                                nc.tensor.matmul(ps[:, ts(n, 512)],
                                                 w2[:, k, :],
                                                 gT[:, k, ts(n, 512)],
                                                 start=(k == 0), stop=(k == FK - 1))
                        nc.vector.tensor_scalar(ps[:], ps[:],
                                                b2_sb[:, m:m + 1], g2_sb[:, m:m + 1],
                                                op0=ALU.add, op1=ALU.mult)
                        nc.vector.tensor_add(resid[:, m, :], ps[:], resid[:, m, :])
                        nc.sync.dma_start(outT[ts(m, 128), :], resid[:, m, :])

    nc.compile()
    return nc


_CACHE = {}


def _get_runner():
    """Build nc once and return (nc, jitted shard_map callable, in_names, out_shape)."""
    if "runner" in _CACHE:
        return _CACHE["runner"]
    import jax
    from jax.sharding import Mesh, PartitionSpec
    from jax.experimental.shard_map import shard_map
    from concourse import bass2jax, mybir as _mb

    nc = _build()
    bass2jax.install_neuronx_cc_hook()

    partition_name = nc.partition_id_tensor.name if nc.partition_id_tensor else None
    in_names, out_names, out_avals = [], [], []
    for alloc in nc.m.functions[0].allocations:
        if not isinstance(alloc, _mb.MemoryLocationSet):
            continue
        name = alloc.memorylocations[0].name
        if alloc.kind == "ExternalInput":
            if name != partition_name:
                in_names.append(name)
        elif alloc.kind == "ExternalOutput":
            out_names.append(name)
            out_avals.append(jax.core.ShapedArray(tuple(alloc.tensor_shape),
                                                  _mb.dt.np(alloc.dtype)))
    n_params = len(in_names)
    all_names = list(in_names) + list(out_names)
    if partition_name is not None:
        all_names.append(partition_name)

    def _body(*args):
        operands = list(args)
        if partition_name is not None:
            operands.append(bass2jax.partition_id_tensor())
        outs = bass2jax._bass_exec_p.bind(
            *operands,
            out_avals=tuple(out_avals),
            in_names=tuple(all_names),
            out_names=tuple(out_names),
            lowering_input_output_aliases=(),
            sim_require_finite=True,
            sim_require_nnan=True,
            nc=nc,
        )
        return tuple(outs)

    devices = jax.devices()[:N_CORES]
    mesh = Mesh(np.asarray(devices), ("core",))
    sharded = jax.jit(shard_map(
        _body, mesh=mesh,
        in_specs=(PartitionSpec("core"),) * (n_params + len(out_names)),
        out_specs=(PartitionSpec("core"),) * len(out_names),
        check_rep=False))
    _CACHE["runner"] = (nc, sharded, in_names, out_names, out_avals)
    return _CACHE["runner"]


def _prep_in_maps(inputs):
    x = np.asarray(inputs["x"], dtype=np.float32)          # [8, 1024, 768]
    base = {
        "qkv_w": np.asarray(inputs["qkv_w"]).astype(bf16),
        "proj_w": np.asarray(inputs["proj_w"]).astype(bf16),
        "fc1_w": np.asarray(inputs["fc1_w"]).astype(bf16),
        "fc2_w": np.asarray(inputs["fc2_w"]).astype(bf16),
        "qkv_b": np.asarray(inputs["qkv_b"], dtype=np.float32),
        "proj_b": np.asarray(inputs["proj_b"], dtype=np.float32),
        "fc1_b": np.asarray(inputs["fc1_b"], dtype=np.float32),
        "fc2_b": np.asarray(inputs["fc2_b"], dtype=np.float32),
        "ln1_w": np.asarray(inputs["ln1_w"], dtype=np.float32),
        "ln1_b": np.asarray(inputs["ln1_b"], dtype=np.float32),
        "ln2_w": np.asarray(inputs["ln2_w"], dtype=np.float32),
        "ln2_b": np.asarray(inputs["ln2_b"], dtype=np.float32),
        "gamma1": np.asarray(inputs["gamma1"], dtype=np.float32),
        "gamma2": np.asarray(inputs["gamma2"], dtype=np.float32),
    }
    in_maps = []
    for i in range(N_CORES):
        m = dict(base)
        m["xT"] = np.ascontiguousarray(x[i].T)
        in_maps.append(m)
    return in_maps


def _stage_args(in_maps):
    """Concatenate per-core inputs along axis 0 (shard_map layout) + zero outs."""
    import jax
    nc, sharded, in_names, out_names, out_avals = _get_runner()
    args = [np.concatenate([np.asarray(m[name]) for m in in_maps], axis=0)
            for name in in_names]
    for av in out_avals:
        args.append(np.zeros((N_CORES * av.shape[0],) + av.shape[1:], av.dtype))
    return [jax.device_put(a) for a in args]


def _run(args):
    _, sharded, _, out_names, out_avals = _get_runner()
    outs = sharded(*args)
    return outs


def kernel(**inputs) -> np.ndarray:
    in_maps = _prep_in_maps(inputs)
    args = _stage_args(in_maps)
    outs = _run(args)
    outT = np.asarray(outs[0]).reshape(N_CORES, C, T)      # [8, 768, 1024]
    return np.ascontiguousarray(outT.transpose(0, 2, 1)).astype(np.float32)


# revision 21
# speedup vs baseline: 1.0872x; 1.0872x over previous
"""Transformer block (pre-LN, 12-head attention + GELU MLP) on 8 TRN2 NeuronCores.

Sharding: pure data-parallel — batch 8 -> one sequence [1024, 768] per core,
no collectives. On-device everything is computed in "feature-major" layout
(features on partitions, tokens on the free axis) so no on-device transposes
are needed anywhere:

  - x arrives host-pre-transposed as xT [768, 1024] f32 (the f32 residual spine).
  - LayerNorm stats (over features = partitions) via ones-vector matmuls on PE;
    normalization applied with partition-broadcast rows on DVE.
  - qkv produces Q^T/K^T feature-major and V token-major directly; scores are
    computed as S^T = K^T.T-free matmuls (contraction over head dim), softmax
    without max-subtraction (logits are bounded ~|S|<4); the 1/8 scale is
    folded into exp's affine pre-scale; the softmax denominator comes from a
    ones-column appended to V, and normalization is applied to O^T (64 rows)
    fused into the PSUM->SBUF copy.
  - All matmul inputs in bf16 (weights pre-cast on host), f32 PSUM accumulate.
  - Output written feature-major, transposed back on host.
"""

import numpy as np
import ml_dtypes

import concourse.bass as bass
import concourse.tile as tile
from concourse import bacc, mybir
from concourse.bass import ts

F32 = mybir.dt.float32
BF16 = mybir.dt.bfloat16
AF = mybir.ActivationFunctionType
ALU = mybir.AluOpType

T = 1024          # tokens per core
C = 768           # model dim
H = 12            # heads
DH = 64           # head dim
FF = 3072         # mlp hidden
CK = C // 128     # 6 feature chunks
FK = FF // 128    # 24 hidden chunks
TK = T // 128     # 8 token tiles
NT = T // 512     # 2 psum-width chunks
EPS = 1e-5
N_CORES = 8

bf16 = ml_dtypes.bfloat16


def _emit_layernorm(nc, tc, ctx_pools, src, dst, lnw_sb, lnb_sb, ones_sb, eps_sb):
    """src: [128, CK, T] f32 sbuf tile; dst: [128, CK, T] bf16 sbuf tile.
    lnw_sb/lnb_sb: [128, CK] f32 (per-partition scalar columns per chunk)."""
    cast_pool, stat_pool, bc_pool = ctx_pools
    with tc.tile_pool(name="ln_psum", bufs=1, space="PSUM") as pst:
        sum_ps = pst.tile([1, T], F32, tag="sum")
        ssq_ps = pst.tile([1, T], F32, tag="ssq")
        for j in range(CK):
            xb = cast_pool.tile([128, T], BF16, tag="xb")
            nc.vector.tensor_copy(xb[:], src[:, j, :])
            sq = cast_pool.tile([128, T], BF16, tag="sq")
            nc.vector.tensor_mul(sq[:], xb[:], xb[:])
            for n in range(NT):
                nc.tensor.matmul(sum_ps[:, ts(n, 512)], ones_sb[:], xb[:, ts(n, 512)],
                                 start=(j == 0), stop=(j == CK - 1))
                nc.tensor.matmul(ssq_ps[:, ts(n, 512)], ones_sb[:], sq[:, ts(n, 512)],
                                 start=(j == 0), stop=(j == CK - 1))
        # stats on partition 0 rows [1, T]
        mu = stat_pool.tile([1, T], F32, tag="mu")
        nc.scalar.mul(mu[:], sum_ps[:], 1.0 / C)
        e2 = stat_pool.tile([1, T], F32, tag="e2")
        nc.scalar.mul(e2[:], ssq_ps[:], 1.0 / C)
    var = stat_pool.tile([1, T], F32, tag="var")
    nc.vector.tensor_mul(var[:], mu[:], mu[:])       # mu^2
    nc.vector.tensor_sub(var[:], e2[:], var[:])      # E[x^2] - mu^2
    nc.scalar.activation(var[:], var[:], AF.Sqrt, bias=eps_sb[:])  # std, in place
    rstd = stat_pool.tile([1, T], F32, tag="rstd")
    nc.vector.reciprocal(rstd[:], var[:])
    nc.vector.tensor_mul(mu[:], mu[:], rstd[:])      # mu * rstd, in place
    s_bc = bc_pool.tile([128, T], F32, tag="s_bc")
    nc.gpsimd.partition_broadcast(s_bc[:], rstd[:])
    m_bc = bc_pool.tile([128, T], F32, tag="m_bc")
    nc.gpsimd.partition_broadcast(m_bc[:], mu[:])
    for j in range(CK):
        t1 = cast_pool.tile([128, T], F32, tag="n1")
        nc.vector.tensor_mul(t1[:], src[:, j, :], s_bc[:])   # x * rstd
        nc.vector.tensor_sub(t1[:], t1[:], m_bc[:])          # - mu*rstd
        nc.vector.tensor_scalar(dst[:, j, :], t1[:],
                                lnw_sb[:, j:j + 1], lnb_sb[:, j:j + 1],
                                op0=ALU.mult, op1=ALU.add)


def _build():
    nc = bacc.Bacc("TRN2", target_bir_lowering=False, debug=False)

    xT = nc.dram_tensor("xT", [C, T], F32, kind="ExternalInput").ap()
    qkv_w = nc.dram_tensor("qkv_w", [C, 3 * C], BF16, kind="ExternalInput").ap()
    proj_w = nc.dram_tensor("proj_w", [C, C], BF16, kind="ExternalInput").ap()
    fc1_w = nc.dram_tensor("fc1_w", [C, FF], BF16, kind="ExternalInput").ap()
    # fc2 host-pre-tiled: row (m*128+p), col (k*128+c) = fc2_w[k*128+p, m*128+c]
    fc2_wt = nc.dram_tensor("fc2_wt", [C, FF], BF16, kind="ExternalInput").ap()
    # all [C]/[3C]/[FF] vectors host-packed as per-partition columns [128, 90]:
    # ln1w 0:6 | ln1b 6:12 | ln2w 12:18 | ln2b 18:24 | g1 24:30 | g2 30:36 |
    # qkv_b 36:54 | proj_b 54:60 | fc1_b 60:84 | fc2_b 84:90
    vpack = nc.dram_tensor("vpack", [128, 90], F32, kind="ExternalInput").ap()
    vbias = nc.dram_tensor("vbias", [C], F32, kind="ExternalInput").ap()  # qkv_b v-part
    outT = nc.dram_tensor("outT", [C, T], F32, kind="ExternalOutput").ap()

    with tile.TileContext(nc) as tc:
        import contextlib
        with contextlib.ExitStack() as ctx:
            spine = ctx.enter_context(tc.tile_pool(name="spine", bufs=1))
            hTp = ctx.enter_context(tc.tile_pool(name="hTp", bufs=1))
            vecs = ctx.enter_context(tc.tile_pool(name="vecs", bufs=1))
            consts = ctx.enter_context(tc.tile_pool(name="consts", bufs=1))
            cast_pool = ctx.enter_context(tc.tile_pool(name="cast", bufs=2))
            stat_pool = ctx.enter_context(tc.tile_pool(name="stat", bufs=1))
            bc_pool = ctx.enter_context(tc.tile_pool(name="bc", bufs=1))
            ln_pools = (cast_pool, stat_pool, bc_pool)

            # ---- small persistent constants & vectors ----
            ones_sb = consts.tile([128, 1], BF16)
            nc.vector.memset(ones_sb[:], 1.0)
            eps_sb = consts.tile([1, 1], F32)
            nc.vector.memset(eps_sb[:], EPS)

            vp = vecs.tile([128, 90], F32, tag="vpack")
            nc.sync.dma_start(vp[:], vpack[:])
            ln1w_sb, ln1b_sb = vp[:, 0:6], vp[:, 6:12]
            ln2w_sb, ln2b_sb = vp[:, 12:18], vp[:, 18:24]
            g1_sb, g2_sb = vp[:, 24:30], vp[:, 30:36]
            qb_sb = vp[:, 36:54]
            pb_sb = vp[:, 54:60]
            b1_sb = vp[:, 60:84]
            b2_sb = vp[:, 84:90]
            # v-bias broadcast along partitions: [128, C] f32
            vb_bc = vecs.tile([128, C], F32, tag="vb_bc")
            vb_src = bass.AP(tensor=vbias.tensor, offset=vbias.offset,
                             ap=[[0, 128]] + vbias[:].ap)
            nc.sync.dma_start(vb_bc[:], vb_src)

            # ---- load x spine (in-place evolves x -> x1 -> out) ----
            resid = spine.tile([128, CK, T], F32, tag="spine")
            for j in range(CK):
                nc.sync.dma_start(resid[:, j, :], xT[ts(j, 128), :])

            # ---- LN1 ----
            h1T = hTp.tile([128, CK, T], BF16, tag="hT")
            _emit_layernorm(nc, tc, ln_pools, resid, h1T, ln1w_sb, ln1b_sb, ones_sb, eps_sb)

            with contextlib.ExitStack() as actx:
                wsec = actx.enter_context(tc.tile_pool(name="wsec", bufs=2))
                qtp = actx.enter_context(tc.tile_pool(name="qtp", bufs=1))
                ktp = actx.enter_context(tc.tile_pool(name="ktp", bufs=1))
                vtp = actx.enter_context(tc.tile_pool(name="vtp", bufs=1))
                otp = actx.enter_context(tc.tile_pool(name="otp", bufs=1))
                esp = actx.enter_context(tc.tile_pool(name="esp", bufs=2))
                zip_ = actx.enter_context(tc.tile_pool(name="zip", bufs=1))
                zbp = actx.enter_context(tc.tile_pool(name="zbp", bufs=2))

                QT = qtp.tile([128, CK, T], BF16)
                KT = ktp.tile([128, CK, T], BF16)
                V = vtp.tile([128, TK, H, DH + 1], BF16)
                OT = otp.tile([128, CK, T], BF16)

                # ---- qkv matmuls ----
                with tc.tile_pool(name="qkv_ps", bufs=2, space="PSUM") as mmp:
                    for sec in range(2):  # 0 -> Q^T, 1 -> K^T  (feature-major)
                        w = wsec.tile([128, CK, C], BF16, tag="wsec")
                        for k in range(CK):
                            nc.sync.dma_start(w[:, k, :],
                                              qkv_w[ts(k, 128), sec * C:(sec + 1) * C])
                        dst = QT if sec == 0 else KT
                        for m in range(CK):
                            ps = mmp.tile([128, T], F32, tag="mm")
                            for k in range(CK):
                                for n in range(NT):
                                    nc.tensor.matmul(ps[:, ts(n, 512)],
                                                     w[:, k, ts(m, 128)],
                                                     h1T[:, k, ts(n, 512)],
                                                     start=(k == 0), stop=(k == CK - 1))
                            nc.vector.tensor_scalar(dst[:, m, :], ps[:],
                                                    qb_sb[:, sec * CK + m:sec * CK + m + 1],
                                                    None, op0=ALU.add)
                    # V token-major
                    w = wsec.tile([128, CK, C], BF16, tag="wsec")
                    for k in range(CK):
                        nc.sync.dma_start(w[:, k, :], qkv_w[ts(k, 128), 2 * C:3 * C])
                    nc.vector.memset(V[:, :, :, DH:DH + 1], 1.0)
                    for tk in range(TK):
                        ps = mmp.tile([128, C], F32, tag="vmm")
                        for k in range(CK):
                            nc.tensor.matmul(ps[:, 0:512], h1T[:, k, ts(tk, 128)],
                                             w[:, k, 0:512],
                                             start=(k == 0), stop=(k == CK - 1))
                            nc.tensor.matmul(ps[:, 512:C], h1T[:, k, ts(tk, 128)],
                                             w[:, k, 512:C],
                                             start=(k == 0), stop=(k == CK - 1))
                        nc.vector.tensor_add(
                            V[:, tk, :, 0:DH],
                            ps.rearrange("p (h d) -> p h d", d=DH),
                            vb_bc.rearrange("p (h d) -> p h d", d=DH))

                # ---- attention ----
                with tc.tile_pool(name="s_ps", bufs=2, space="PSUM") as spp, \
                     tc.tile_pool(name="o_ps", bufs=2, space="PSUM") as opp:
                    for h in range(H):
                        zq = 64 * (h % 2)
                        jq = h // 2
                        es = esp.tile([128, TK, T], BF16, tag="es")
                        for tk in range(TK):
                            sp = spp.tile([128, T], F32, tag="sp")
                            for n in range(NT):
                                nc.tensor.matmul(
                                    sp[:, ts(n, 512)],
                                    KT[zq:zq + 64, jq, ts(tk, 128)],
                                    QT[zq:zq + 64, jq, ts(n, 512)])
                            nc.scalar.activation(es[:, tk, :], sp[:], AF.Exp,
                                                 scale=float(DH) ** -0.5)
                        op = opp.tile([DH + 1, T], F32, tag="op")
                        for tk in range(TK):
                            for n in range(NT):
                                nc.tensor.matmul(op[:, ts(n, 512)],
                                                 V[:, tk, h, :],
                                                 es[:, tk, ts(n, 512)],
                                                 start=(tk == 0), stop=(tk == TK - 1))
                        zi = zip_.tile([1, T], F32, tag="zi")
                        nc.vector.reciprocal(zi[:], op[DH:DH + 1, :])
                        zbc = zbp.tile([64, T], F32, tag="zbc")
                        nc.gpsimd.partition_broadcast(zbc[:], zi[:])
                        nc.vector.tensor_mul(OT[zq:zq + 64, jq, :], op[0:DH, :], zbc[:])

                # ---- proj + residual -> x1 (in place on resid) ----
                w = wsec.tile([128, CK, C], BF16, tag="wsec")
                for k in range(CK):
                    nc.sync.dma_start(w[:, k, :], proj_w[ts(k, 128), :])
                with tc.tile_pool(name="pj_ps", bufs=2, space="PSUM") as pjp:
                    for m in range(CK):
                        ps = pjp.tile([128, T], F32, tag="pj")
                        for k in range(CK):
                            for n in range(NT):
                                nc.tensor.matmul(ps[:, ts(n, 512)],
                                                 w[:, k, ts(m, 128)],
                                                 OT[:, k, ts(n, 512)],
                                                 start=(k == 0), stop=(k == CK - 1))
                        # (ps + proj_b) * gamma1, then + xT
                        nc.vector.tensor_scalar(ps[:], ps[:],
                                                pb_sb[:, m:m + 1], g1_sb[:, m:m + 1],
                                                op0=ALU.add, op1=ALU.mult)
                        nc.vector.tensor_add(resid[:, m, :], ps[:], resid[:, m, :])

            # ---- LN2 ----
            h2T = hTp.tile([128, CK, T], BF16, tag="hT")
            _emit_layernorm(nc, tc, ln_pools, resid, h2T, ln2w_sb, ln2b_sb, ones_sb, eps_sb)

            # ---- MLP ----
            with contextlib.ExitStack() as mctx:
                w1p = mctx.enter_context(tc.tile_pool(name="w1p", bufs=1))
                gtp = mctx.enter_context(tc.tile_pool(name="gtp", bufs=1))
                w2p = mctx.enter_context(tc.tile_pool(name="w2p", bufs=2))

                gT = gtp.tile([128, FK, T], BF16)
                w1 = w1p.tile([128, CK, FF], BF16)
                for k in range(CK):
                    nc.sync.dma_start(w1[:, k, :], fc1_w[ts(k, 128), :])

                with tc.tile_pool(name="f1_ps", bufs=2, space="PSUM") as f1p, \
                     tc.tile_pool(name="f2_ps", bufs=2, space="PSUM") as f2p:
                    for m in range(FK):
                        ps = f1p.tile([128, T], F32, tag="f1")
                        for k in range(CK):
                            for n in range(NT):
                                nc.tensor.matmul(ps[:, ts(n, 512)],
                                                 w1[:, k, ts(m, 128)],
                                                 h2T[:, k, ts(n, 512)],
                                                 start=(k == 0), stop=(k == CK - 1))
                        nc.scalar.activation(gT[:, m, :], ps[:], AF.Gelu,
                                             bias=b1_sb[:, m:m + 1])
                    for m in range(CK):
                        w2 = w2p.tile([128, FK, 128], BF16, tag="w2")
                        nc.sync.dma_start(
                            w2.rearrange("p k c -> p (k c)"),
                            fc2_wt[ts(m, 128), :])
                        ps = f2p.tile([128, T], F32, tag="f2")
                        for k in range(FK):
                            for n in range(NT):
                                nc.tensor.matmul(ps[:, ts(n, 512)],
                                                 w2[:, k, :],
                                                 gT[:, k, ts(n, 512)],
                                                 start=(k == 0), stop=(k == FK - 1))
                        nc.vector.tensor_scalar(ps[:], ps[:],
                                                b2_sb[:, m:m + 1], g2_sb[:, m:m + 1],
                                                op0=ALU.add, op1=ALU.mult)
                        nc.vector.tensor_add(resid[:, m, :], ps[:], resid[:, m, :])
                        nc.sync.dma_start(outT[ts(m, 128), :], resid[:, m, :])

    nc.compile()
    return nc


_CACHE = {}


def _get_runner():
    """Build nc once and return (nc, jitted shard_map callable, in_names, out_shape)."""
    if "runner" in _CACHE:
        return _CACHE["runner"]
    import jax
    from jax.sharding import Mesh, PartitionSpec
    from jax.experimental.shard_map import shard_map
    from concourse import bass2jax, mybir as _mb

    nc = _build()
    bass2jax.install_neuronx_cc_hook()

    partition_name = nc.partition_id_tensor.name if nc.partition_id_tensor else None
    in_names, out_names, out_avals = [], [], []
    for alloc in nc.m.functions[0].allocations:
        if not isinstance(alloc, _mb.MemoryLocationSet):
            continue
        name = alloc.memorylocations[0].name
        if alloc.kind == "ExternalInput":
            if name != partition_name:
                in_names.append(name)
        elif alloc.kind == "ExternalOutput":
            out_names.append(name)
            out_avals.append(jax.core.ShapedArray(tuple(alloc.tensor_shape),
                                                  _mb.dt.np(alloc.dtype)))
    n_params = len(in_names)
    all_names = list(in_names) + list(out_names)
    if partition_name is not None:
        all_names.append(partition_name)

    def _body(*args):
        operands = list(args)
        if partition_name is not None:
            operands.append(bass2jax.partition_id_tensor())
        outs = bass2jax._bass_exec_p.bind(
            *operands,
            out_avals=tuple(out_avals),
            in_names=tuple(all_names),
            out_names=tuple(out_names),
            lowering_input_output_aliases=(),
            sim_require_finite=True,
            sim_require_nnan=True,
            nc=nc,
        )
        return tuple(outs)

    devices = jax.devices()[:N_CORES]
    mesh = Mesh(np.asarray(devices), ("core",))
    sharded = jax.jit(shard_map(
        _body, mesh=mesh,
        in_specs=(PartitionSpec("core"),) * (n_params + len(out_names)),
        out_specs=(PartitionSpec("core"),) * len(out_names),
        check_rep=False))
    _CACHE["runner"] = (nc, sharded, in_names, out_names, out_avals)
    return _CACHE["runner"]


def _prep_in_maps(inputs):
    x = np.asarray(inputs["x"], dtype=np.float32)          # [8, 1024, 768]
    f32 = lambda k: np.asarray(inputs[k], dtype=np.float32)

    def cols(v):  # [n*128] -> [128, n] per-partition columns
        return np.ascontiguousarray(v.reshape(-1, 128).T)

    vpack = np.concatenate([
        cols(f32("ln1_w")), cols(f32("ln1_b")),
        cols(f32("ln2_w")), cols(f32("ln2_b")),
        cols(f32("gamma1")), cols(f32("gamma2")),
        cols(f32("qkv_b")), cols(f32("proj_b")),
        cols(f32("fc1_b")), cols(f32("fc2_b")),
    ], axis=1)                                             # [128, 90]

    fc2 = np.asarray(inputs["fc2_w"]).astype(bf16)          # [3072, 768]
    # tile so that row (m*128+p), col (k*128+c) = fc2_w[k*128+p, m*128+c]
    fc2_wt = np.ascontiguousarray(
        fc2.reshape(FK, 128, CK, 128).transpose(2, 1, 0, 3).reshape(C, FF))

    base = {
        "qkv_w": np.asarray(inputs["qkv_w"]).astype(bf16),
        "proj_w": np.asarray(inputs["proj_w"]).astype(bf16),
        "fc1_w": np.asarray(inputs["fc1_w"]).astype(bf16),
        "fc2_wt": fc2_wt,
        "vpack": vpack,
        "vbias": np.ascontiguousarray(f32("qkv_b")[2 * C:3 * C]),
    }
    in_maps = []
    for i in range(N_CORES):
        m = dict(base)
        m["xT"] = np.ascontiguousarray(x[i].T)
        in_maps.append(m)
    return in_maps


def _stage_args(in_maps):
    """Concatenate per-core inputs along axis 0 (shard_map layout) + zero outs."""
    import jax
    nc, sharded, in_names, out_names, out_avals = _get_runner()
    args = [np.concatenate([np.asarray(m[name]) for m in in_maps], axis=0)
            for name in in_names]
    for av in out_avals:
        args.append(np.zeros((N_CORES * av.shape[0],) + av.shape[1:], av.dtype))
    return [jax.device_put(a) for a in args]


def _run(args):
    _, sharded, _, out_names, out_avals = _get_runner()
    outs = sharded(*args)
    return outs


def kernel(**inputs) -> np.ndarray:
    in_maps = _prep_in_maps(inputs)
    args = _stage_args(in_maps)
    outs = _run(args)
    outT = np.asarray(outs[0]).reshape(N_CORES, C, T)      # [8, 768, 1024]
    return np.ascontiguousarray(outT.transpose(0, 2, 1)).astype(np.float32)
